# revision 24
# baseline (speedup 1.0000x reference)
"""RX(theta) on qubit 5 of a [B=4, 2^24] complex state (real/imag split), int8.

The rel-err gate (2e-2, max-abs-normalized) admits int8 transport with a
single global scale: quantization error is uniform-absolute (~1.1e-2 on this
data), unlike fp8 whose relative grid fails near the max. int8 halves HBM
traffic vs the f16 kernel: 32 MiB per core total (16.8 in + 16.8 out).

Device pipeline per core (units of 2-8K columns over a [128, 131072] stream):
  load int8 (SP ring) -> DVE unpack int8->f16 (2x_2P mode, one instr/unit)
  -> PE: 4x4 block-diagonal rotation matmul, f16 weights, 512-col matmuls
     into [128, 1024] f32 PSUM chunks (4 chunks in flight = 8 banks; with
     only 2 larger chunks the next unit's first matmul WARs on the evict
     stream and costs ~1.1us/unit)
  -> evict PSUM -> int8 SBUF with round-to-nearest; chunks split between
     ACT (6/unit) and DVE (last 2 per 8K unit, deferred until after the
     next unit's unpack) so both engines run gapless and ~equally busy
     (DVE ~103us: unpacks 70 + evicts 33; ACT ~102us)
  -> store int8 (ACT ring; tail stores alternate rings).

Units are 2048/4096 cols at the head and 2048 at the tail (shorter serial
chain during pipeline ramp/drain) and 8192 in steady state. A dummy ACT op
up front pulls the Copy activation-table DMA into the startup window.

Layout: partition p = 4*(slab_local*2 + half) + comp, comp in (r0, r1, i0,
i1) - the RX pair-update is then a [128,128] block-diag matmul along
partitions, and every DMA is contiguous per partition (4-8 KiB runs).

Scales: sig_in = M/125 (M = max abs input), sig_out = (|c|+|s|)*sig_in*1.012;
weights are f16(c*sig_in/sig_out) etc., so PSUM values are the output int8
codes directly; the f32->int8 write conversion rounds-to-nearest (verified
on HW). Host de/quantizes and permutes; theta enters only via the weights.

Measured: ~129 us/core in the chip's fast state, ~152 us when the shared
chip lands in its ~1.2x-throttled state (uniform clock/HBM slowdown, not
controllable from the kernel; both states ~1.5x faster than the 197/236 us
of the f16 ancestor of this kernel).
"""

import os
import sys

import numpy as np

if "CONCOURSE_ROOT" not in os.environ:
    try:
        import concourse  # noqa: F401
    except ImportError:
        sys.path.insert(0, "/opt/trn_rl_repo")

from concourse import bacc, bass  # noqa: F401
from concourse.bass_utils import run_bass_kernel_spmd
from concourse.tile import TileContext
import concourse.mybir as mybir

# bass_utils' trace path does `from antenv.axon_hooks import ...`; some images
# lack that submodule, which would crash a BASS_TRACE=1 run. Register a stub so
# tracing degrades to a warning instead (a harness may install the real hook
# before importing this module).
try:
    import antenv.axon_hooks  # noqa: F401
except ImportError:
    import types as _types

    import antenv as _antenv

    _hooks = _types.ModuleType("antenv.axon_hooks")
    _hooks._hook = None
    _hooks.set_axon_ntff_profile_hook = lambda h: setattr(_hooks, "_hook", h)
    _hooks.get_axon_ntff_profile_hook = lambda: _hooks._hook
    sys.modules["antenv.axon_hooks"] = _hooks
    _antenv.axon_hooks = _hooks

B = 4
NQ = 24
QUBIT = 5
DIM = 2**NQ
N_CORES = 8
P = 128
TOTC = 131072  # columns per core
CHC = 1024  # cols per PSUM chunk
MMC = 512  # cols per matmul
JJ = 2**17
F16 = mybir.dt.float16
F32 = mybir.dt.float32
I8 = mybir.dt.int8

# unit widths: small at the edges for pipeline ramp/drain, big in the middle
UNITS = [1024, 2048, 4096, 4096] + [8192] * 14 + [2048, 1024, 1024, 1024]
assert sum(UNITS) == TOTC

_PROGRAM_CACHE: dict = {}
LAST_RESULTS = None  # BassKernelResults of the most recent run (for harness)


def build_program(
    units: tuple = tuple(UNITS),
    q_bufs: int = 6,
    x_bufs: int = 4,
    y_bufs: int = 3,
    psum_bufs: int = 4,
    load_engine: str = "sync",
    store_engine: str = "scalar",
    w_engine: str = "gpsimd",
    dve_evict_min_w: int = 8192,  # units this wide evict their last chunk on DVE
):
    nc = bacc.Bacc(None)
    xq = nc.dram_tensor("xq", [P, TOTC], I8, kind="ExternalInput")
    wT = nc.dram_tensor("wT", [P, P], F16, kind="ExternalInput")
    yq = nc.dram_tensor("yq", [P, TOTC], I8, kind="ExternalOutput")

    with TileContext(nc) as tc:
        with (
            tc.tile_pool(name="w", bufs=1) as wpool,
            tc.tile_pool(name="q", bufs=q_bufs) as qpool,
            tc.tile_pool(name="x", bufs=x_bufs) as xpool,
            tc.tile_pool(name="y", bufs=y_bufs) as ypool,
            tc.tile_pool(name="ps", bufs=psum_bufs, space="PSUM") as pspool,
        ):
            tw = wpool.tile([P, P], F16)

            ld = getattr(nc, load_engine)
            st = getattr(nc, store_engine)

            # dummy ACT op up front so the Copy activation-table DMA
            # (~1.5us) overlaps the ring bring-up instead of gating the
            # first real eviction
            warm = wpool.tile([P, 1], F16)
            nc.vector.memset(warm[:], 0.0)
            nc.scalar.copy(out=warm[:], in_=warm[:])

            ntail = 4  # last units: alternate evict engines, per-chunk stores
            pending = []  # deferred (evict, store) emitters from previous unit

            off = 0
            for t, w in enumerate(units):
                q = qpool.tile([P, w], I8, name=f"q{t}", tag="q")
                ld.dma_start(out=q[:], in_=xq[:, off : off + w])
                if t == 0:
                    # after the first load so it does not delay it
                    getattr(nc, w_engine).dma_start(out=tw[:], in_=wT[:])
                xb = xpool.tile([P, w], F16, name=f"x{t}", tag="x")
                nc.vector.tensor_scalar_mul(out=xb[:], in0=q[:], scalar1=1.0)
                for fn in pending:
                    fn()
                pending = []
                y = ypool.tile([P, w], I8, name=f"y{t}", tag="y")
                nchunks = w // CHC
                tail = t >= len(units) - ntail
                uoff = off
                # with psum_bufs=4 slots of 1024 cols, the slot the NEXT
                # unit's first matmul needs is freed by a mid-stream evict
                # of this unit (4 allocations back), hidden behind the later
                # evicts -- so the engine assignment is latency-uncritical.
                # DVE takes the last 2 chunks of big units (load balance),
                # deferred past the next unit's unpack.
                # DVE helps with the last 2 chunks of big units, minus a
                # couple of units so DVE and ACT finish together
                dve_set = (
                    {nchunks - 1} if t in (5, 11) else set(range(nchunks - 2, nchunks))
                ) if not tail and w >= dve_evict_min_w else set()
                deferred = []
                for ch in range(nchunks):
                    pt = pspool.tile([P, CHC], F32, name=f"p{t}_{ch}", tag="p")
                    for j in range(CHC // MMC):
                        cs = slice(ch * CHC + j * MMC, ch * CHC + (j + 1) * MMC)
                        nc.tensor.matmul(
                            pt[:, j * MMC : (j + 1) * MMC],
                            tw[:],
                            xb[:, cs],
                            start=True,
                            stop=True,
                        )
                    ys = y[:, ch * CHC : (ch + 1) * CHC]
                    if tail:
                        # drain: alternate engines and DMA rings (loads are
                        # done, so the load ring is free), store per chunk
                        if (t + ch) % 2 == 0:
                            nc.scalar.copy(out=ys, in_=pt[:])
                        else:
                            nc.vector.tensor_scalar_mul(out=ys, in0=pt[:], scalar1=1.0)
                        ring = st if (t + ch) % 2 == 0 else ld
                        ring.dma_start(
                            out=yq[:, uoff + ch * CHC : uoff + (ch + 1) * CHC], in_=ys
                        )
                    elif ch in dve_set:
                        deferred.append((pt, ys))
                    else:
                        nc.scalar.copy(out=ys, in_=pt[:])
                if deferred:

                    def emit(deferred=deferred, uoff=uoff, w=w, y=y):
                        for pt, ys in deferred:
                            nc.vector.tensor_scalar_mul(out=ys, in0=pt[:], scalar1=1.0)
                        st.dma_start(out=yq[:, uoff : uoff + w], in_=y[:])

                    pending.append(emit)
                elif not tail:
                    st.dma_start(out=yq[:, off : off + w], in_=y[:])
                off += w
            for fn in pending:
                fn()
    nc.finalize()
    return nc


def _get_program(key="default", **kwargs):
    if key not in _PROGRAM_CACHE:
        _PROGRAM_CACHE[key] = build_program(**kwargs)
    return _PROGRAM_CACHE[key]


def _kernel_numpy(state_real, state_imag, theta, qubit, num_qubits):
    """Fallback for shapes/params the Bass program wasn't built for."""
    b = state_real.shape[0]
    left = 2**qubit
    right = 2 ** (num_qubits - qubit - 1)
    r = state_real.reshape(b, left, 2, right)
    im = state_imag.reshape(b, left, 2, right)
    half = np.float32(theta[0]) * np.float32(0.5)
    c = np.cos(half, dtype=np.float32)
    s = np.sin(half, dtype=np.float32)
    r0, r1 = r[:, :, 0], r[:, :, 1]
    i0, i1 = im[:, :, 0], im[:, :, 1]
    nr0 = c * r0 + s * i1
    ni0 = c * i0 - s * r1
    nr1 = c * r1 + s * i0
    ni1 = c * i1 - s * r0
    out_r = np.stack([nr0, nr1], axis=2).reshape(b, -1).astype(np.float32)
    out_i = np.stack([ni0, ni1], axis=2).reshape(b, -1).astype(np.float32)
    return out_r, out_i


def _to_device_layout(qr, qi):
    """int8 [B, DIM] x2 -> [N_CORES, 128, TOTC] with partition layout
    p = 4*(slab_local*2 + half) + comp, comp = (r0, r1, i0, i1)."""
    r4 = qr.reshape(B, 32, 2, 2, JJ)  # (b, l, h, hf, jj)
    i4 = qi.reshape(B, 32, 2, 2, JJ)
    # (b, l, hf, comp, jj)
    a = np.stack([r4[:, :, 0], r4[:, :, 1], i4[:, :, 0], i4[:, :, 1]], axis=3)
    # (slab, hf, comp, jj) -> partitions are (sl, hf, comp)
    a = np.ascontiguousarray(a.reshape(N_CORES, P, JJ))
    return a


def kernel(state_real, state_imag, theta, qubit=QUBIT, num_qubits=NQ):
    global LAST_RESULTS
    state_real = np.asarray(state_real, dtype=np.float32)
    state_imag = np.asarray(state_imag, dtype=np.float32)
    theta = np.asarray(theta, dtype=np.float32)

    if (
        int(qubit) != QUBIT
        or int(num_qubits) != NQ
        or state_real.shape != (B, DIM)
        or state_imag.shape != (B, DIM)
    ):
        return _kernel_numpy(state_real, state_imag, theta, int(qubit), int(num_qubits))

    half = float(theta[0]) * 0.5
    c = np.cos(half)
    s = np.sin(half)
    amp = abs(c) + abs(s)
    M = float(max(np.abs(state_real).max(), np.abs(state_imag).max()))
    if M == 0.0 or amp == 0.0:
        return _kernel_numpy(state_real, state_imag, theta, int(qubit), int(num_qubits))
    sig_in = M / 125.0
    sig_out = amp * sig_in * 1.012
    wc = np.float16(c * sig_in / sig_out)
    ws = np.float16(s * sig_in / sig_out)

    R = np.array(
        [
            [wc, 0, 0, ws],
            [0, wc, ws, 0],
            [0, -ws, wc, 0],
            [-ws, 0, 0, wc],
        ],
        dtype=np.float32,
    )
    W = np.zeros((P, P), dtype=np.float32)
    for g in range(32):
        W[4 * g : 4 * g + 4, 4 * g : 4 * g + 4] = R
    wT = W.T.astype(np.float16)

    inv = np.float32(1.0 / sig_in)
    qr = np.clip(np.rint(state_real * inv), -127, 127).astype(np.int8)
    qi = np.clip(np.rint(state_imag * inv), -127, 127).astype(np.int8)
    xdev = _to_device_layout(qr, qi)

    nc = _get_program()
    in_maps = [{"xq": xdev[k], "wT": wT} for k in range(N_CORES)]
    res = run_bass_kernel_spmd(nc, in_maps, list(range(N_CORES)))
    LAST_RESULTS = res

    y = np.stack([res.results[k]["yq"] for k in range(N_CORES)])
    aa = y.reshape(B, 32, 2, 4, JJ)  # (b, l, hf, comp, jj)
    so = np.float32(sig_out)
    out_r = np.empty((B, DIM), dtype=np.float32)
    out_i = np.empty((B, DIM), dtype=np.float32)
    vr = out_r.reshape(B, 32, 2, 2, JJ)  # (b, l, h, hf, jj)
    vi = out_i.reshape(B, 32, 2, 2, JJ)
    vr[:, :, 0] = aa[:, :, :, 0] * so
    vr[:, :, 1] = aa[:, :, :, 1] * so
    vi[:, :, 0] = aa[:, :, :, 2] * so
    vi[:, :, 1] = aa[:, :, :, 3] * so
    return out_r, out_i


# revision 27
# speedup vs baseline: 1.0231x; 1.0231x over previous
"""RX(theta) on qubit 5 of a [B=4, 2^24] complex state (real/imag split), int8.

The rel-err gate (2e-2, max-abs-normalized) admits int8 transport with a
single global scale: quantization error is uniform-absolute (~1.1e-2 on this
data), unlike fp8 whose relative grid fails near the max. int8 halves HBM
traffic vs the f16 kernel: 32 MiB per core total (16.8 in + 16.8 out).

Device pipeline per core (units of 2-8K columns over a [128, 131072] stream):
  load int8 (SP ring) -> DVE unpack int8->f16 (2x_2P mode, one instr/unit)
  -> PE: 4x4 block-diagonal rotation matmul, f16 weights, 512-col matmuls
     into [128, 1024] f32 PSUM chunks (4 chunks in flight = 8 banks; with
     only 2 larger chunks the next unit's first matmul WARs on the evict
     stream and costs ~1.1us/unit)
  -> evict PSUM -> int8 SBUF with round-to-nearest; chunks split between
     ACT (6/unit) and DVE (last 2 per 8K unit, deferred until after the
     next unit's unpack) so both engines run gapless and ~equally busy
     (DVE ~103us: unpacks 70 + evicts 33; ACT ~102us)
  -> store int8 (ACT ring; tail stores alternate rings).

Units are 2048/4096 cols at the head and 2048 at the tail (shorter serial
chain during pipeline ramp/drain) and 8192 in steady state. A dummy ACT op
up front pulls the Copy activation-table DMA into the startup window.

Layout: partition p = 4*(slab_local*2 + half) + comp, comp in (r0, r1, i0,
i1) - the RX pair-update is then a [128,128] block-diag matmul along
partitions, and every DMA is contiguous per partition (4-8 KiB runs).

Scales: sig_in = M/125 (M = max abs input), sig_out = (|c|+|s|)*sig_in*1.012;
weights are f16(c*sig_in/sig_out) etc., so PSUM values are the output int8
codes directly; the f32->int8 write conversion rounds-to-nearest (verified
on HW). Host de/quantizes and permutes; theta enters only via the weights.

Measured: ~129 us/core in the chip's fast state, ~152 us when the shared
chip lands in its ~1.2x-throttled state (uniform clock/HBM slowdown, not
controllable from the kernel; both states ~1.5x faster than the 197/236 us
of the f16 ancestor of this kernel).
"""

import os
import sys

import numpy as np

if "CONCOURSE_ROOT" not in os.environ:
    try:
        import concourse  # noqa: F401
    except ImportError:
        sys.path.insert(0, "/opt/trn_rl_repo")

from concourse import bacc, bass  # noqa: F401
from concourse.bass_utils import run_bass_kernel_spmd
from concourse.tile import TileContext
import concourse.mybir as mybir

# bass_utils' trace path does `from antenv.axon_hooks import ...`; some images
# lack that submodule, which would crash a BASS_TRACE=1 run. Register a stub so
# tracing degrades to a warning instead (a harness may install the real hook
# before importing this module).
try:
    import antenv.axon_hooks  # noqa: F401
except ImportError:
    import types as _types

    import antenv as _antenv

    _hooks = _types.ModuleType("antenv.axon_hooks")
    _hooks._hook = None
    _hooks.set_axon_ntff_profile_hook = lambda h: setattr(_hooks, "_hook", h)
    _hooks.get_axon_ntff_profile_hook = lambda: _hooks._hook
    sys.modules["antenv.axon_hooks"] = _hooks
    _antenv.axon_hooks = _hooks

B = 4
NQ = 24
QUBIT = 5
DIM = 2**NQ
N_CORES = 8
P = 128
TOTC = 131072  # columns per core
CHC = 1024  # cols per PSUM chunk
MMC = 512  # cols per matmul
JJ = 2**17
F16 = mybir.dt.float16
F32 = mybir.dt.float32
I8 = mybir.dt.int8

# unit widths: small at the edges for pipeline ramp/drain, big in the middle
UNITS = [2048, 4096, 4096] + [8192] * 14 + [2048, 2048, 2048]
assert sum(UNITS) == TOTC

_PROGRAM_CACHE: dict = {}
LAST_RESULTS = None  # BassKernelResults of the most recent run (for harness)


def build_program(
    units: tuple = tuple(UNITS),
    q_bufs: int = 6,
    x_bufs: int = 3,
    y_bufs: int = 3,
    psum_bufs: int = 4,
    load_engine: str = "sync",
    store_engine: str = "scalar",
    w_engine: str = "gpsimd",
    dve_evict_min_w: int = 8192,  # units this wide evict their last chunk on DVE
):
    nc = bacc.Bacc(None)
    xq = nc.dram_tensor("xq", [P, TOTC], I8, kind="ExternalInput")
    wT = nc.dram_tensor("wT", [P, P], F16, kind="ExternalInput")
    yq = nc.dram_tensor("yq", [P, TOTC], I8, kind="ExternalOutput")

    with TileContext(nc) as tc:
        with (
            tc.tile_pool(name="w", bufs=1) as wpool,
            tc.tile_pool(name="q", bufs=q_bufs) as qpool,
            tc.tile_pool(name="x", bufs=x_bufs) as xpool,
            tc.tile_pool(name="y", bufs=y_bufs) as ypool,
            tc.tile_pool(name="ps", bufs=psum_bufs, space="PSUM") as pspool,
        ):
            tw = wpool.tile([P, P], F16)

            ld = getattr(nc, load_engine)
            st = getattr(nc, store_engine)

            # dummy ACT op up front so the Copy activation-table DMA
            # (~1.5us) overlaps the ring bring-up instead of gating the
            # first real eviction
            warm = wpool.tile([P, 1], F16)
            nc.vector.memset(warm[:], 0.0)
            nc.scalar.copy(out=warm[:], in_=warm[:])

            ntail = 3  # last units: alternate evict engines, per-chunk stores
            pending = []  # deferred (evict, store) emitters from previous unit

            off = 0
            for t, w in enumerate(units):
                q = qpool.tile([P, w], I8, name=f"q{t}", tag="q")
                ld.dma_start(out=q[:], in_=xq[:, off : off + w])
                if t == 0:
                    # after the first load so it does not delay it
                    getattr(nc, w_engine).dma_start(out=tw[:], in_=wT[:])
                xb = xpool.tile([P, w], F16, name=f"x{t}", tag="x")
                nc.vector.tensor_scalar_mul(out=xb[:], in0=q[:], scalar1=1.0)
                for fn in pending:
                    fn()
                pending = []
                y = ypool.tile([P, w], I8, name=f"y{t}", tag="y")
                nchunks = w // CHC
                tail = t >= len(units) - ntail
                uoff = off
                # with psum_bufs=4 slots of 1024 cols, the slot the NEXT
                # unit's first matmul needs is freed by a mid-stream evict
                # of this unit (4 allocations back), hidden behind the later
                # evicts -- so the engine assignment is latency-uncritical.
                # DVE takes the last 2 chunks of big units (load balance),
                # deferred past the next unit's unpack.
                # DVE helps with the last 2 chunks of big units, minus a
                # couple of units so DVE and ACT finish together
                dve_set = (
                    {nchunks - 1} if t in (5, 11) else set(range(nchunks - 2, nchunks))
                ) if not tail and w >= dve_evict_min_w else set()
                deferred = []
                for ch in range(nchunks):
                    pt = pspool.tile([P, CHC], F32, name=f"p{t}_{ch}", tag="p")
                    for j in range(CHC // MMC):
                        cs = slice(ch * CHC + j * MMC, ch * CHC + (j + 1) * MMC)
                        nc.tensor.matmul(
                            pt[:, j * MMC : (j + 1) * MMC],
                            tw[:],
                            xb[:, cs],
                            start=True,
                            stop=True,
                        )
                    ys = y[:, ch * CHC : (ch + 1) * CHC]
                    if tail:
                        # drain: alternate engines and DMA rings (loads are
                        # done, so the load ring is free), store per chunk
                        if (t + ch) % 2 == 0:
                            nc.scalar.copy(out=ys, in_=pt[:])
                        else:
                            nc.vector.tensor_scalar_mul(out=ys, in0=pt[:], scalar1=1.0)
                        ring = st if (t + ch) % 2 == 0 else ld
                        ring.dma_start(
                            out=yq[:, uoff + ch * CHC : uoff + (ch + 1) * CHC], in_=ys
                        )
                    elif ch in dve_set:
                        deferred.append((pt, ys))
                    else:
                        nc.scalar.copy(out=ys, in_=pt[:])
                if deferred:

                    def emit(deferred=deferred, uoff=uoff, w=w, y=y):
                        for pt, ys in deferred:
                            nc.vector.tensor_scalar_mul(out=ys, in0=pt[:], scalar1=1.0)
                        st.dma_start(out=yq[:, uoff : uoff + w], in_=y[:])

                    pending.append(emit)
                elif not tail:
                    st.dma_start(out=yq[:, off : off + w], in_=y[:])
                off += w
            for fn in pending:
                fn()
    nc.finalize()
    return nc


def _get_program(key="default", **kwargs):
    if key not in _PROGRAM_CACHE:
        _PROGRAM_CACHE[key] = build_program(**kwargs)
    return _PROGRAM_CACHE[key]


def _kernel_numpy(state_real, state_imag, theta, qubit, num_qubits):
    """Fallback for shapes/params the Bass program wasn't built for."""
    b = state_real.shape[0]
    left = 2**qubit
    right = 2 ** (num_qubits - qubit - 1)
    r = state_real.reshape(b, left, 2, right)
    im = state_imag.reshape(b, left, 2, right)
    half = np.float32(theta[0]) * np.float32(0.5)
    c = np.cos(half, dtype=np.float32)
    s = np.sin(half, dtype=np.float32)
    r0, r1 = r[:, :, 0], r[:, :, 1]
    i0, i1 = im[:, :, 0], im[:, :, 1]
    nr0 = c * r0 + s * i1
    ni0 = c * i0 - s * r1
    nr1 = c * r1 + s * i0
    ni1 = c * i1 - s * r0
    out_r = np.stack([nr0, nr1], axis=2).reshape(b, -1).astype(np.float32)
    out_i = np.stack([ni0, ni1], axis=2).reshape(b, -1).astype(np.float32)
    return out_r, out_i


def _to_device_layout(qr, qi):
    """int8 [B, DIM] x2 -> [N_CORES, 128, TOTC] with partition layout
    p = 4*(slab_local*2 + half) + comp, comp = (r0, r1, i0, i1)."""
    r4 = qr.reshape(B, 32, 2, 2, JJ)  # (b, l, h, hf, jj)
    i4 = qi.reshape(B, 32, 2, 2, JJ)
    # (b, l, hf, comp, jj)
    a = np.stack([r4[:, :, 0], r4[:, :, 1], i4[:, :, 0], i4[:, :, 1]], axis=3)
    # (slab, hf, comp, jj) -> partitions are (sl, hf, comp)
    a = np.ascontiguousarray(a.reshape(N_CORES, P, JJ))
    return a


def kernel(state_real, state_imag, theta, qubit=QUBIT, num_qubits=NQ):
    global LAST_RESULTS
    state_real = np.asarray(state_real, dtype=np.float32)
    state_imag = np.asarray(state_imag, dtype=np.float32)
    theta = np.asarray(theta, dtype=np.float32)

    if (
        int(qubit) != QUBIT
        or int(num_qubits) != NQ
        or state_real.shape != (B, DIM)
        or state_imag.shape != (B, DIM)
    ):
        return _kernel_numpy(state_real, state_imag, theta, int(qubit), int(num_qubits))

    half = float(theta[0]) * 0.5
    c = np.cos(half)
    s = np.sin(half)
    amp = abs(c) + abs(s)
    M = float(max(np.abs(state_real).max(), np.abs(state_imag).max()))
    if M == 0.0 or amp == 0.0:
        return _kernel_numpy(state_real, state_imag, theta, int(qubit), int(num_qubits))
    sig_in = M / 125.0
    sig_out = amp * sig_in * 1.012
    wc = np.float16(c * sig_in / sig_out)
    ws = np.float16(s * sig_in / sig_out)

    R = np.array(
        [
            [wc, 0, 0, ws],
            [0, wc, ws, 0],
            [0, -ws, wc, 0],
            [-ws, 0, 0, wc],
        ],
        dtype=np.float32,
    )
    W = np.zeros((P, P), dtype=np.float32)
    for g in range(32):
        W[4 * g : 4 * g + 4, 4 * g : 4 * g + 4] = R
    wT = W.T.astype(np.float16)

    inv = np.float32(1.0 / sig_in)
    qr = np.clip(np.rint(state_real * inv), -127, 127).astype(np.int8)
    qi = np.clip(np.rint(state_imag * inv), -127, 127).astype(np.int8)
    xdev = _to_device_layout(qr, qi)

    nc = _get_program()
    in_maps = [{"xq": xdev[k], "wT": wT} for k in range(N_CORES)]
    res = run_bass_kernel_spmd(nc, in_maps, list(range(N_CORES)))
    LAST_RESULTS = res

    y = np.stack([res.results[k]["yq"] for k in range(N_CORES)])
    aa = y.reshape(B, 32, 2, 4, JJ)  # (b, l, hf, comp, jj)
    so = np.float32(sig_out)
    out_r = np.empty((B, DIM), dtype=np.float32)
    out_i = np.empty((B, DIM), dtype=np.float32)
    vr = out_r.reshape(B, 32, 2, 2, JJ)  # (b, l, h, hf, jj)
    vi = out_i.reshape(B, 32, 2, 2, JJ)
    vr[:, :, 0] = aa[:, :, :, 0] * so
    vr[:, :, 1] = aa[:, :, :, 1] * so
    vi[:, :, 0] = aa[:, :, :, 2] * so
    vi[:, :, 1] = aa[:, :, :, 3] * so
    return out_r, out_i


# revision 28
# speedup vs baseline: 1.0513x; 1.0276x over previous
"""RX(theta) on qubit 5 of a [B=4, 2^24] complex state (real/imag split), int8.

The rel-err gate (2e-2, max-abs-normalized) admits int8 transport with a
single global scale: quantization error is uniform-absolute (~1.1e-2 on this
data), unlike fp8 whose relative grid fails near the max. int8 halves HBM
traffic vs the f16 kernel: 32 MiB per core total (16.8 in + 16.8 out).

Device pipeline per core (units of 2-8K columns over a [128, 131072] stream):
  load int8 (SP ring) -> DVE unpack int8->f16 (2x_2P mode, one instr/unit)
  -> PE: 4x4 block-diagonal rotation matmul, f16 weights, 512-col matmuls
     into [128, 1024] f32 PSUM chunks (4 chunks in flight = 8 banks; with
     only 2 larger chunks the next unit's first matmul WARs on the evict
     stream and costs ~1.1us/unit)
  -> evict PSUM -> int8 SBUF with round-to-nearest; chunks split between
     ACT (6/unit) and DVE (last 2 per 8K unit, deferred until after the
     next unit's unpack) so both engines run gapless and ~equally busy
     (DVE ~103us: unpacks 70 + evicts 33; ACT ~102us)
  -> store int8 (ACT ring; tail stores alternate rings).

Units are 2048/4096 cols at the head and 2048 at the tail (shorter serial
chain during pipeline ramp/drain) and 8192 in steady state. A dummy ACT op
up front pulls the Copy activation-table DMA into the startup window.

Layout: partition p = 4*(slab_local*2 + half) + comp, comp in (r0, r1, i0,
i1) - the RX pair-update is then a [128,128] block-diag matmul along
partitions, and every DMA is contiguous per partition (4-8 KiB runs).

Scales: sig_in = M/125 (M = max abs input), sig_out = (|c|+|s|)*sig_in*1.012;
weights are f16(c*sig_in/sig_out) etc., so PSUM values are the output int8
codes directly; the f32->int8 write conversion rounds-to-nearest (verified
on HW). Host de/quantizes and permutes; theta enters only via the weights.

Measured: ~129 us/core in the chip's fast state, ~152 us when the shared
chip lands in its ~1.2x-throttled state (uniform clock/HBM slowdown, not
controllable from the kernel; both states ~1.5x faster than the 197/236 us
of the f16 ancestor of this kernel).
"""

import os
import sys

import numpy as np

if "CONCOURSE_ROOT" not in os.environ:
    try:
        import concourse  # noqa: F401
    except ImportError:
        sys.path.insert(0, "/opt/trn_rl_repo")

from concourse import bacc, bass  # noqa: F401
from concourse.bass_utils import run_bass_kernel_spmd
from concourse.tile import TileContext
import concourse.mybir as mybir

# bass_utils' trace path does `from antenv.axon_hooks import ...`; some images
# lack that submodule, which would crash a BASS_TRACE=1 run. Register a stub so
# tracing degrades to a warning instead (a harness may install the real hook
# before importing this module).
try:
    import antenv.axon_hooks  # noqa: F401
except ImportError:
    import types as _types

    import antenv as _antenv

    _hooks = _types.ModuleType("antenv.axon_hooks")
    _hooks._hook = None
    _hooks.set_axon_ntff_profile_hook = lambda h: setattr(_hooks, "_hook", h)
    _hooks.get_axon_ntff_profile_hook = lambda: _hooks._hook
    sys.modules["antenv.axon_hooks"] = _hooks
    _antenv.axon_hooks = _hooks

B = 4
NQ = 24
QUBIT = 5
DIM = 2**NQ
N_CORES = 8
P = 128
TOTC = 131072  # columns per core
CHC = 1024  # cols per PSUM chunk
MMC = 512  # cols per matmul
JJ = 2**17
F16 = mybir.dt.float16
F32 = mybir.dt.float32
I8 = mybir.dt.int8

# unit widths: small at the edges for pipeline ramp/drain, big in the middle
UNITS = [2048, 4096, 4096] + [8192] * 14 + [2048, 2048, 2048]
assert sum(UNITS) == TOTC

_PROGRAM_CACHE: dict = {}
LAST_RESULTS = None  # BassKernelResults of the most recent run (for harness)


def build_program(
    units: tuple = tuple(UNITS),
    q_bufs: int = 6,
    x_bufs: int = 3,
    y_bufs: int = 3,
    psum_bufs: int = 4,
    load_engine: str = "sync",
    store_engine: str = "gpsimd",
    w_engine: str = "gpsimd",
    dve_evict_min_w: int = 8192,  # units this wide evict their last chunk on DVE
):
    nc = bacc.Bacc(None)
    xq = nc.dram_tensor("xq", [P, TOTC], I8, kind="ExternalInput")
    wT = nc.dram_tensor("wT", [P, P], F16, kind="ExternalInput")
    yq = nc.dram_tensor("yq", [P, TOTC], I8, kind="ExternalOutput")

    with TileContext(nc) as tc:
        with (
            tc.tile_pool(name="w", bufs=1) as wpool,
            tc.tile_pool(name="q", bufs=q_bufs) as qpool,
            tc.tile_pool(name="x", bufs=x_bufs) as xpool,
            tc.tile_pool(name="y", bufs=y_bufs) as ypool,
            tc.tile_pool(name="ps", bufs=psum_bufs, space="PSUM") as pspool,
        ):
            tw = wpool.tile([P, P], F16)

            ld = getattr(nc, load_engine)
            st = getattr(nc, store_engine)

            # dummy ACT op up front so the Copy activation-table DMA
            # (~1.5us) overlaps the ring bring-up instead of gating the
            # first real eviction
            warm = wpool.tile([P, 1], F16)
            nc.vector.memset(warm[:], 0.0)
            nc.scalar.copy(out=warm[:], in_=warm[:])

            ntail = 3  # last units: alternate evict engines, per-chunk stores
            pending = []  # deferred (evict, store) emitters from previous unit

            off = 0
            for t, w in enumerate(units):
                q = qpool.tile([P, w], I8, name=f"q{t}", tag="q")
                ld.dma_start(out=q[:], in_=xq[:, off : off + w])
                if t == 0:
                    # after the first load so it does not delay it
                    getattr(nc, w_engine).dma_start(out=tw[:], in_=wT[:])
                xb = xpool.tile([P, w], F16, name=f"x{t}", tag="x")
                nc.vector.tensor_scalar_mul(out=xb[:], in0=q[:], scalar1=1.0)
                for fn in pending:
                    fn()
                pending = []
                y = ypool.tile([P, w], I8, name=f"y{t}", tag="y")
                nchunks = w // CHC
                tail = t >= len(units) - ntail
                uoff = off
                # with psum_bufs=4 slots of 1024 cols, the slot the NEXT
                # unit's first matmul needs is freed by a mid-stream evict
                # of this unit (4 allocations back), hidden behind the later
                # evicts -- so the engine assignment is latency-uncritical.
                # DVE takes the last 2 chunks of big units (load balance),
                # deferred past the next unit's unpack.
                # DVE helps with the last 2 chunks of big units, minus a
                # couple of units so DVE and ACT finish together
                dve_set = (
                    {nchunks - 1} if t in (5, 11) else set(range(nchunks - 2, nchunks))
                ) if not tail and w >= dve_evict_min_w else set()
                deferred = []
                for ch in range(nchunks):
                    pt = pspool.tile([P, CHC], F32, name=f"p{t}_{ch}", tag="p")
                    for j in range(CHC // MMC):
                        cs = slice(ch * CHC + j * MMC, ch * CHC + (j + 1) * MMC)
                        nc.tensor.matmul(
                            pt[:, j * MMC : (j + 1) * MMC],
                            tw[:],
                            xb[:, cs],
                            start=True,
                            stop=True,
                        )
                    ys = y[:, ch * CHC : (ch + 1) * CHC]
                    if tail:
                        # drain: alternate engines and DMA rings (loads are
                        # done, so the load ring is free), store per chunk
                        if (t + ch) % 2 == 0:
                            nc.scalar.copy(out=ys, in_=pt[:])
                        else:
                            nc.vector.tensor_scalar_mul(out=ys, in0=pt[:], scalar1=1.0)
                        ring = st if (t + ch) % 2 == 0 else ld
                        ring.dma_start(
                            out=yq[:, uoff + ch * CHC : uoff + (ch + 1) * CHC], in_=ys
                        )
                    elif ch in dve_set:
                        deferred.append((pt, ys))
                    else:
                        nc.scalar.copy(out=ys, in_=pt[:])
                if deferred:

                    def emit(deferred=deferred, uoff=uoff, w=w, y=y):
                        for pt, ys in deferred:
                            nc.vector.tensor_scalar_mul(out=ys, in0=pt[:], scalar1=1.0)
                        st.dma_start(out=yq[:, uoff : uoff + w], in_=y[:])

                    pending.append(emit)
                elif not tail:
                    st.dma_start(out=yq[:, off : off + w], in_=y[:])
                off += w
            for fn in pending:
                fn()
    nc.finalize()
    return nc


def _get_program(key="default", **kwargs):
    if key not in _PROGRAM_CACHE:
        _PROGRAM_CACHE[key] = build_program(**kwargs)
    return _PROGRAM_CACHE[key]


def _kernel_numpy(state_real, state_imag, theta, qubit, num_qubits):
    """Fallback for shapes/params the Bass program wasn't built for."""
    b = state_real.shape[0]
    left = 2**qubit
    right = 2 ** (num_qubits - qubit - 1)
    r = state_real.reshape(b, left, 2, right)
    im = state_imag.reshape(b, left, 2, right)
    half = np.float32(theta[0]) * np.float32(0.5)
    c = np.cos(half, dtype=np.float32)
    s = np.sin(half, dtype=np.float32)
    r0, r1 = r[:, :, 0], r[:, :, 1]
    i0, i1 = im[:, :, 0], im[:, :, 1]
    nr0 = c * r0 + s * i1
    ni0 = c * i0 - s * r1
    nr1 = c * r1 + s * i0
    ni1 = c * i1 - s * r0
    out_r = np.stack([nr0, nr1], axis=2).reshape(b, -1).astype(np.float32)
    out_i = np.stack([ni0, ni1], axis=2).reshape(b, -1).astype(np.float32)
    return out_r, out_i


def _to_device_layout(qr, qi):
    """int8 [B, DIM] x2 -> [N_CORES, 128, TOTC] with partition layout
    p = 4*(slab_local*2 + half) + comp, comp = (r0, r1, i0, i1)."""
    r4 = qr.reshape(B, 32, 2, 2, JJ)  # (b, l, h, hf, jj)
    i4 = qi.reshape(B, 32, 2, 2, JJ)
    # (b, l, hf, comp, jj)
    a = np.stack([r4[:, :, 0], r4[:, :, 1], i4[:, :, 0], i4[:, :, 1]], axis=3)
    # (slab, hf, comp, jj) -> partitions are (sl, hf, comp)
    a = np.ascontiguousarray(a.reshape(N_CORES, P, JJ))
    return a


def kernel(state_real, state_imag, theta, qubit=QUBIT, num_qubits=NQ):
    global LAST_RESULTS
    state_real = np.asarray(state_real, dtype=np.float32)
    state_imag = np.asarray(state_imag, dtype=np.float32)
    theta = np.asarray(theta, dtype=np.float32)

    if (
        int(qubit) != QUBIT
        or int(num_qubits) != NQ
        or state_real.shape != (B, DIM)
        or state_imag.shape != (B, DIM)
    ):
        return _kernel_numpy(state_real, state_imag, theta, int(qubit), int(num_qubits))

    half = float(theta[0]) * 0.5
    c = np.cos(half)
    s = np.sin(half)
    amp = abs(c) + abs(s)
    M = float(max(np.abs(state_real).max(), np.abs(state_imag).max()))
    if M == 0.0 or amp == 0.0:
        return _kernel_numpy(state_real, state_imag, theta, int(qubit), int(num_qubits))
    sig_in = M / 125.0
    sig_out = amp * sig_in * 1.012
    wc = np.float16(c * sig_in / sig_out)
    ws = np.float16(s * sig_in / sig_out)

    R = np.array(
        [
            [wc, 0, 0, ws],
            [0, wc, ws, 0],
            [0, -ws, wc, 0],
            [-ws, 0, 0, wc],
        ],
        dtype=np.float32,
    )
    W = np.zeros((P, P), dtype=np.float32)
    for g in range(32):
        W[4 * g : 4 * g + 4, 4 * g : 4 * g + 4] = R
    wT = W.T.astype(np.float16)

    inv = np.float32(1.0 / sig_in)
    qr = np.clip(np.rint(state_real * inv), -127, 127).astype(np.int8)
    qi = np.clip(np.rint(state_imag * inv), -127, 127).astype(np.int8)
    xdev = _to_device_layout(qr, qi)

    nc = _get_program()
    in_maps = [{"xq": xdev[k], "wT": wT} for k in range(N_CORES)]
    res = run_bass_kernel_spmd(nc, in_maps, list(range(N_CORES)))
    LAST_RESULTS = res

    y = np.stack([res.results[k]["yq"] for k in range(N_CORES)])
    aa = y.reshape(B, 32, 2, 4, JJ)  # (b, l, hf, comp, jj)
    so = np.float32(sig_out)
    out_r = np.empty((B, DIM), dtype=np.float32)
    out_i = np.empty((B, DIM), dtype=np.float32)
    vr = out_r.reshape(B, 32, 2, 2, JJ)  # (b, l, h, hf, jj)
    vi = out_i.reshape(B, 32, 2, 2, JJ)
    vr[:, :, 0] = aa[:, :, :, 0] * so
    vr[:, :, 1] = aa[:, :, :, 1] * so
    vi[:, :, 0] = aa[:, :, :, 2] * so
    vi[:, :, 1] = aa[:, :, :, 3] * so
    return out_r, out_i
